# revision 9
# baseline (speedup 1.0000x reference)
"""CoordAtt Trainium2 Bass kernel (v3).

Reference computation (per batch n, c=256, h=w=64, mip=8):
    xs   = x + residual                      (bilinear resize of residual at
                                              identical shape is the identity)
    y    = concat(mean_w(xs), mean_h(xs))    -> [mip, h+w]
    y    = hswish(BN(w1 @ y + b1))           -> [mip, h+w]
    a_h  = sigmoid(w2 @ y[:, :h] + b2)       -> [c, h]
    a_w  = sigmoid(w3 @ y[:, h:] + b3)       -> [c, w]
    out  = 2*xs*a_h*a_w + 2*residual*(1 - a_h*a_w)
         = (2*a_h)*a_w*x + 2*residual        (algebraically identical)

Design (8 cores, data-parallel over batch n: 2 batches/core). The kernel is
HBM-bound: 24 MiB/core mandatory traffic at 360 B/ns ~= 70 us, so everything
else is scheduled to hide under the DMA stream:

  * conv-before-pool in float32r (fp32 at full PE rate, no bf16 casts):
    y = w1'^T @ x + w1'^T @ res via PSUM accumulation, where w1' has the
    folded BN scale (gamma/sqrt(var+eps)/W) pre-multiplied during on-chip
    weight prep (transpose matmul against diag(scale)).  The conv output is
    PARTITION-STACKED: the stationary for quarter q is w1'^T replicated 4x
    into 32-column block q of a [c, 64] tile (zeros elsewhere), so PSUM
    accumulation of the two quarters lands each h-quarter in its own
    32-partition block of a [64, 1024] psum tile per half.  The
    directional pools then reduce a free dim of 1024 instead of 4096 (4x
    less DVE time).  Identity column-slice matmuls (built on-chip via
    affine_select) gather pooled values back to [mip, h+w] layout.
  * hswish folds to 3 ops: u = relu(y + bias+3); v = (u-3) * min(u,6)/6.
  * weights are DMA'd contiguously, interleaved into the sync-queue load
    stream so their HWDGE descriptor slots never starve the DMA engines.
  * finals: t = x*ah (f32->bf16), t *= 2*aw (bf16: DVE 2x mode), out =
    2r + t, stored at half-tile granularity.  GpSimd has no fused
    scalar_tensor_tensor on hardware, so its units use Act (o=2r) + a
    GpSimd add; engine assignment and emission order are hand-tuned so
    batch 1's output stream meets the DMA slots right after its loads.
  * hardware ISA quirks found the hard way: fp32r matmuls may only write
    dst partition base 0 (hence the masked stationaries), and any matmul
    operand at partition base 32 kills the device (hence identity
    column-slice gathers instead of partition-sliced stationaries).
"""

import numpy as np

import concourse.bacc as bacc
import concourse.mybir as mybir
from concourse.tile import TileContext
from concourse.bass_utils import run_bass_kernel_spmd

F32 = mybir.dt.float32
F32R = mybir.dt.float32r
BF16 = mybir.dt.bfloat16
Alu = mybir.AluOpType
Act = mybir.ActivationFunctionType
AX = mybir.AxisListType

N_CORES = 8
N, C, H, W = 16, 256, 64, 64
NLOC = N // N_CORES           # batches per core
MIP = 8
EPS = 1e-5
HW = H * W                    # 4096 flat (h w) columns per (batch, chunk)
NCHUNK = C // 128             # channel chunks (2)
HCOL = HW // 2                # 2048 columns per half tile
QH = H // 4                   # h rows per quarter (16)
QCOL = QH * W                 # columns per quarter (1024)


def build_module():
    nc = bacc.Bacc("TRN2", target_bir_lowering=False)

    x_d = nc.dram_tensor("x", (NLOC, C, H, W), F32, kind="ExternalInput")
    r_d = nc.dram_tensor("residual", (NLOC, C, H, W), F32, kind="ExternalInput")
    w1_d = nc.dram_tensor("w1", (MIP, C), F32, kind="ExternalInput")
    b1_d = nc.dram_tensor("b1", (MIP,), F32, kind="ExternalInput")
    gamma_d = nc.dram_tensor("bn_gamma", (MIP,), F32, kind="ExternalInput")
    beta_d = nc.dram_tensor("bn_beta", (MIP,), F32, kind="ExternalInput")
    mean_d = nc.dram_tensor("bn_mean", (MIP,), F32, kind="ExternalInput")
    var_d = nc.dram_tensor("bn_var", (MIP,), F32, kind="ExternalInput")
    w2_d = nc.dram_tensor("w2", (C, MIP), F32, kind="ExternalInput")
    b2_d = nc.dram_tensor("b2", (C,), F32, kind="ExternalInput")
    w3_d = nc.dram_tensor("w3", (C, MIP), F32, kind="ExternalInput")
    b3_d = nc.dram_tensor("b3", (C,), F32, kind="ExternalInput")
    out_d = nc.dram_tensor("out", (NLOC, C, H, W), F32, kind="ExternalOutput")

    with TileContext(nc) as tc:
        with (
            tc.tile_pool(name="big", bufs=1) as big,
            tc.tile_pool(name="tbf", bufs=1) as tbf,
            tc.tile_pool(name="small", bufs=1) as small,
            tc.tile_pool(name="work", bufs=2) as work,
            tc.tile_pool(name="psum_y", bufs=1, space="PSUM") as psum_y,
            tc.tile_pool(name="psum_mlp", bufs=1, space="PSUM") as psum_mlp,
            tc.tile_pool(name="psum_att", bufs=1, space="PSUM") as psum_att,
        ):
            # ---- input loads + weight DMAs, all on the sync queue so HWDGE
            # grants follow this order and the DMA engines never starve;
            # tiny weight transfers slot between the 1 MiB input tiles.
            xt = {}
            rt = {}
            w_tiles = {}

            def load(b, k, j):
                js = slice(j * HCOL, (j + 1) * HCOL)
                cs = slice(k * 128, (k + 1) * 128)
                t = big.tile([128, HCOL], F32, name=f"x_{b}_{k}_{j}", tag=f"x{b}{k}{j}")
                nc.sync.dma_start(t[:].bitcast(F32R),
                                  x_d[b, cs].rearrange("c h w -> c (h w)")[:, js].bitcast(F32R))
                xt[b, k, j] = t
                t = big.tile([128, HCOL], F32, name=f"r_{b}_{k}_{j}", tag=f"r{b}{k}{j}")
                nc.sync.dma_start(t[:].bitcast(F32R),
                                  r_d[b, cs].rearrange("c h w -> c (h w)")[:, js].bitcast(F32R))
                rt[b, k, j] = t

            load(0, 0, 0)
            w1f = small.tile([MIP, C], F32, tag="w1f")
            nc.sync.dma_start(w1f[:], w1_d[:, :])
            bn_in = small.tile([MIP, 5], F32, tag="bn_in")
            for i, d in enumerate((var_d, gamma_d, beta_d, mean_d, b1_d)):
                nc.sync.dma_start(bn_in[:, i:i + 1], d[:].unsqueeze(1))
            load(0, 1, 0)
            w2f = small.tile([128, 2 * MIP], F32, tag="w2f")
            nc.sync.dma_start(w2f[:].rearrange("p (k o) -> p k o", k=2),
                              w2_d.rearrange("(k p) o -> p k o", p=128))
            w3f = small.tile([128, 2 * MIP], F32, tag="w3f")
            nc.sync.dma_start(w3f[:].rearrange("p (k o) -> p k o", k=2),
                              w3_d.rearrange("(k p) o -> p k o", p=128))
            b2t = small.tile([128, NCHUNK], F32, tag="b2t")
            nc.sync.dma_start(b2t[:], b2_d.rearrange("(k p) -> p k", p=128))
            b3t = small.tile([128, NCHUNK], F32, tag="b3t")
            nc.sync.dma_start(b3t[:], b3_d.rearrange("(k p) -> p k", p=128))
            for b in range(NLOC):
                for j in range(2):
                    for k in range(NCHUNK):
                        if (b, k, j) not in xt:
                            load(b, k, j)

            # ---- identity / selection matrices (on-chip constants) ----
            ones = small.tile([128, 128], F32, tag="ones")
            nc.vector.memset(ones[:], 1.0)
            ident = small.tile([128, 128], F32, tag="ident")
            nc.gpsimd.affine_select(ident[:], ones[:], [[-1, 128]], Alu.is_equal, 0.0,
                                    base=0, channel_multiplier=1)
            # warm the Act function tables (copy/relu/sqrt set + sigmoid set)
            # during the idle startup so they don't stall the first real ops
            warm = small.tile([MIP, 2], F32, tag="warm")
            nc.vector.memset(warm[:, 0:1], 0.0)
            nc.scalar.copy(warm[:, 1:2], warm[:, 0:1])
            nc.scalar.activation(warm[:, 1:2], warm[:, 0:1], Act.Sigmoid)
            # S32[p, m] = 1 iff p mod 32 == m (m < 8): pool-gather stationary
            s32 = small.tile([128, MIP], F32, tag="s32")
            nc.gpsimd.tensor_tensor(s32[:], ident[:, 0:MIP], ident[:, 32:32 + MIP], Alu.add)
            nc.gpsimd.tensor_tensor(s32[:], s32[:], ident[:, 64:64 + MIP], Alu.add)
            nc.gpsimd.tensor_tensor(s32[:], s32[:], ident[:, 96:96 + MIP], Alu.add)

            # ---- BN fold ----
            # scale_p = gamma/sqrt(var+eps)/W (folded into w1'), bias3_p =
            # (b1-mean)*inv + beta + 3 (the +3 pre-biases relu6(y+3))
            consts = small.tile([MIP, 2], F32, tag="consts")
            nc.vector.memset(consts[:, 0:1], EPS)
            var_c = bn_in[:, 0:1]
            gamma_c = bn_in[:, 1:2]
            beta_c = bn_in[:, 2:3]
            mean_c = bn_in[:, 3:4]
            b1_c = bn_in[:, 4:5]
            bn_t = small.tile([MIP, 4], F32, tag="bn_t")
            sv = bn_t[:, 0:1]
            inv = bn_t[:, 1:2]
            scale_p = bn_t[:, 2:3]
            bias3_p = bn_t[:, 3:4]
            nc.scalar.activation(sv, var_c, Act.Sqrt, bias=consts[:, 0:1], scale=1.0)
            nc.vector.reciprocal(inv, sv)
            nc.vector.tensor_tensor(inv, inv, gamma_c, Alu.mult)
            nc.vector.tensor_scalar_mul(scale_p, inv, 1.0 / W)
            nc.vector.tensor_tensor(bias3_p, b1_c, mean_c, Alu.subtract)
            nc.vector.scalar_tensor_tensor(bias3_p, bias3_p, inv, beta_c, Alu.mult, Alu.add)
            nc.vector.tensor_scalar_add(bias3_p, bias3_p, 3.0)

            zeros = small.tile([128, 1], F32, tag="zeros")
            nc.vector.memset(zeros[:], 0.0)

            # ---- weight prep on PE (plain matmuls against identity/diag) ----
            # ds = diag(scale_p): transpose-matmul against it folds BN scale
            ds = small.tile([MIP, MIP], F32, tag="ds")
            nc.vector.tensor_tensor(ds[:], ident[0:MIP, 0:MIP],
                                    scale_p.broadcast_to((MIP, MIP)), Alu.mult)
            # w1m[k][q]: [c128, 64] masked stationary: (w1^T . diag(scale))
            # chunk k replicated 4x in 32-column block q, zeros in the other
            # block.  PSUM accumulation of the two quarters then yields the
            # partition-stacked conv output with dst partition base 0 (the
            # only base the fp32r matmul ISA allows).
            w1m = []
            for k in range(NCHUNK):
                ps = psum_att.tile([128, MIP], F32, name=f"w1tp{k}", tag=f"att{k}")
                nc.tensor.matmul(ps[:], w1f[:, k * 128:(k + 1) * 128], ds[:],
                                 start=True, stop=True)
                per_q = []
                for q in range(2):
                    t = small.tile([128, 64], F32R, tag=f"w1m_{k}_{q}")
                    nc.scalar.copy(
                        t[:, 32 * (1 - q):32 * (2 - q)],
                        zeros[:].broadcast_to((128, 32)))
                    nc.scalar.copy(
                        t[:, 32 * q:32 * (q + 1)].rearrange("p (a m) -> p a m", a=4),
                        ps[:].unsqueeze(1).broadcast_to((128, 4, MIP)))
                    per_q.append(t)
                w1m.append(per_q)
            # w2t/w3t: [mip, C] via transpose-matmul against I128
            w2t = small.tile([MIP, C], F32, tag="w2t")
            w3t = small.tile([MIP, C], F32, tag="w3t")
            for wt, wf in ((w2t, w2f), (w3t, w3f)):
                for k in range(NCHUNK):
                    ps = psum_mlp.tile([MIP, 128], F32, name=f"wp_{id(wf)}_{k}", tag=f"mlp{k}")
                    nc.tensor.matmul(ps[:], wf[:, k * MIP:(k + 1) * MIP], ident[:, :],
                                     start=True, stop=True)
                    nc.scalar.copy(wt[:, k * 128:(k + 1) * 128], ps[:])

            # ---- per-batch pipeline ----
            # engine assignment per batch for the finals: batch 0 has slack,
            # so its p3s all go to GpSimd (keeping DVE free for batch 1's
            # pool reduces); batch 1's tail is latency-critical, so DVE takes
            # most p3s and the last unit's p1 is emitted late in the finals.
            LAST = NLOC - 1
            for b in range(NLOC):
                p1_eng = {(0, 0): "D", (1, 0): "P", (0, 1): "D", (1, 1): "P"}
                p3_eng = {(0, 0): "D", (1, 0): "P", (0, 1): "D", (1, 1): "P"}
                if b == LAST:
                    p1_late = {(0, 1)}          # emitted inside the finals
                else:
                    p1_late = set()
                    # GpSimd units stored first so batch 1's o-tile reuse
                    # (WAR on these stores) clears early
                    unit_order = [(1, 0), (1, 1), (0, 0), (0, 1)]
                vy = work.tile([MIP, 2 * H], F32, name=f"v_{b}", tag="v")
                mlp_ps = psum_mlp.tile([MIP, 2 * H], F32, name=f"mlp_{b}", tag=f"mlp{b % 2}")
                ah = {}
                aw_bf = {}
                tbf_t = {}
                for k in range(NCHUNK):
                    ah[k] = work.tile([128, H], F32, name=f"ah_{b}_{k}", tag=f"ah_{k}")
                    aw_bf[k] = work.tile([128, W], BF16, name=f"awbf_{b}_{k}", tag=f"awbf_{k}")

                def emit_p1(k, j):
                    t = tbf.tile([128, HCOL], BF16, name=f"t_{b}_{k}_{j}", tag=f"t{k}{j}")
                    tbf_t[k, j] = t
                    xs_ = xt[b, k, j][:].rearrange("p (h w) -> p h w", h=32)
                    ahb = ah[k][:, 32 * j:32 * j + 32].unsqueeze(2).broadcast_to((128, 32, W))
                    tv = t[:].rearrange("p (h w) -> p h w", h=32)
                    eng = nc.vector if p1_eng[k, j] == "D" else nc.gpsimd
                    eng.tensor_tensor(tv, xs_, ahb, Alu.mult)

                def emit_mlp(ps_slice, v_slice, u, m6):
                    # u = relu(y + bias + 3); v = (u-3)*min(u,6)/6 == hswish(BN(y))
                    nc.scalar.activation(u[:], ps_slice, Act.Relu, bias=bias3_p, scale=1.0)
                    nc.vector.tensor_scalar(m6[:], u[:], 6.0, 1.0 / 6.0, Alu.min, Alu.mult)
                    nc.vector.scalar_tensor_tensor(v_slice, u[:], -3.0, m6[:],
                                                   Alu.add, Alu.mult)

                for j in range(2):
                    # conv for half j: psum tile [64, 1024]; quarter q of this
                    # half at partition base 32q; 512-col slices accumulate
                    # x0, r0, x1, r1 (x+res for free).
                    y_ps = psum_y.tile([64, QCOL], F32, name=f"y_{b}_{j}", tag=f"y{j}")
                    srcs = [(0, xt[b, 0, j]), (0, rt[b, 0, j]),
                            (1, xt[b, 1, j]), (1, rt[b, 1, j])]
                    for s in range(2):
                        accum = [(q, k, src) for q in range(2) for (k, src) in srcs]
                        for i, (q, k, src) in enumerate(accum):
                            col = q * QCOL + s * 512
                            nc.tensor.matmul(
                                y_ps[0:64, s * 512:(s + 1) * 512],
                                w1m[k][q][:],
                                src[:, col:col + 512].bitcast(F32R),
                                start=(i == 0), stop=(i == len(accum) - 1),
                            )
                    # directional pools (DVE): free dim is just 1024.  The
                    # a_w path is the batch's critical path, so the column
                    # pool goes first.
                    ywp_s = work.tile([64, W], F32, name=f"ywp_{b}_{j}", tag=f"ywp{j}")
                    nc.vector.reduce_sum(
                        ywp_s[:], y_ps[:].rearrange("p (h w) -> p w h", h=QH), axis=AX.X)
                    # yw gather-sum accumulates across halves
                    nc.tensor.matmul(mlp_ps[:, H:2 * H], s32[0:64, :], ywp_s[:],
                                     start=(j == 0), stop=(j == 1))
                    yh_s = work.tile([64, QH], F32, name=f"yh_{b}_{j}", tag=f"yh{j}")
                    nc.vector.reduce_sum(
                        yh_s[:], y_ps[:].rearrange("p (h w) -> p h w", h=QH), axis=AX.X)
                    if j == 1:
                        # a_w chain as soon as the last column pool lands
                        ws_ = slice(H, 2 * H)
                        uw = work.tile([MIP, W], F32, name=f"uw_{b}", tag="uw")
                        m6w = work.tile([MIP, W], F32, name=f"m6w_{b}", tag="m6w")
                        emit_mlp(mlp_ps[:, ws_], vy[:, ws_], uw, m6w)
                        for k in range(NCHUNK):
                            cs = slice(k * 128, (k + 1) * 128)
                            awp = psum_att.tile([128, W], F32, name=f"awp_{b}_{k}", tag=f"att{k}")
                            nc.tensor.matmul(awp[:], w3t[:, cs], vy[:, ws_],
                                             start=True, stop=True)
                            nc.scalar.activation(aw_bf[k][:], awp[:], Act.Sigmoid,
                                                 bias=b3t[:, k:k + 1], scale=1.0)
                            nc.scalar.mul(aw_bf[k][:], aw_bf[k][:], 2.0)
                    # gather yh quarters -> mlp psum cols [32j, 32j+32)
                    # gather quarter q's rows via an identity column-slice
                    # stationary at base 0 (partition-offset operands are
                    # rejected by the hardware ISA)
                    for q in range(2):
                        nc.tensor.matmul(
                            mlp_ps[:, 32 * j + QH * q: 32 * j + QH * (q + 1)],
                            ident[0:64, 32 * q:32 * q + MIP],
                            yh_s[:, :],
                            start=True, stop=True)
                    # mlp + a_h for this half
                    hs = slice(32 * j, 32 * j + 32)
                    u = work.tile([MIP, 32], F32, name=f"u_{b}_{j}", tag=f"u{j}")
                    m6 = work.tile([MIP, 32], F32, name=f"m6_{b}_{j}", tag=f"m6{j}")
                    emit_mlp(mlp_ps[:, hs], vy[:, hs], u, m6)
                    for k in range(NCHUNK):
                        cs = slice(k * 128, (k + 1) * 128)
                        ahp = psum_att.tile([128, 32], F32, name=f"ahp_{b}_{j}_{k}",
                                            tag=f"att{(2 * j + k) % 2}")
                        nc.tensor.matmul(ahp[:], w2t[:, cs], vy[:, hs], start=True, stop=True)
                        nc.scalar.activation(ah[k][:, hs], ahp[:], Act.Sigmoid,
                                             bias=b2t[:, k:k + 1], scale=1.0)
                    # p1: t = (2x) * ah (f32 -> bf16)
                    for k in range(NCHUNK):
                        if (k, j) not in p1_late:
                            emit_p1(k, j)

                # finals: p2 on DVE (bf16 2x) and p3 + store, at half-tile
                # granularity so the output stream starts right after a_w
                # lands.  The critical batch uses a hand-interleaved order so
                # DVE and GpSimd finish together and every store meets its
                # DMA slot.
                ot = {}
                for j in range(2):
                    for k in range(NCHUNK):
                        # the last batch's outputs reuse batch 0's x buffers
                        # (dead after batch 0's conv+p1) to avoid WAR stalls
                        # against batch 0's outgoing stores
                        tag = f"x0{k}{j}" if b == LAST else f"o{k}{j}"
                        ot[k, j] = big.tile([128, HCOL], F32, name=f"o_{b}_{k}_{j}",
                                            tag=tag)

                def p2_half(k, j, hhalf):
                    t = tbf_t[k, j]
                    sl = slice(hhalf * 1024, (hhalf + 1) * 1024)
                    tv = t[:, sl].rearrange("p (h w) -> p h w", h=16)
                    awb = aw_bf[k][:].unsqueeze(1).broadcast_to((128, 16, W))
                    nc.vector.tensor_tensor(tv, tv, awb, Alu.mult)

                def pre2r(k, j):
                    # o = 2r for GpSimd p3 units (Act has slack; GpSimd lacks
                    # a fused scalar_tensor_tensor on hardware)
                    nc.scalar.mul(ot[k, j][:], rt[b, k, j][:], 2.0)

                def p3_store(k, j, hhalf):
                    t = tbf_t[k, j]
                    r = rt[b, k, j]
                    o = ot[k, j]
                    sl = slice(hhalf * 1024, (hhalf + 1) * 1024)
                    if p3_eng[k, j] == "D":
                        nc.vector.scalar_tensor_tensor(
                            o[:, sl], r[:, sl], 2.0, t[:, sl], Alu.mult, Alu.add)
                    else:
                        nc.gpsimd.tensor_tensor(o[:, sl], o[:, sl], t[:, sl], Alu.add)
                    od = out_d[b, k * 128:(k + 1) * 128].rearrange("c h w -> c (h w)")
                    nc.sync.dma_start(
                        od[:, j * HCOL + sl.start: j * HCOL + sl.stop], o[:, sl])

                for (kk, jj), e in p3_eng.items():
                    if e != "D":
                        pre2r(kk, jj)
                if b == LAST:
                    p2_half(0, 0, 0); p3_store(0, 0, 0)
                    p2_half(0, 0, 1); p3_store(0, 0, 1)
                    p2_half(1, 0, 0); p2_half(1, 0, 1)
                    p3_store(1, 0, 0); p3_store(1, 0, 1)
                    emit_p1(0, 1)
                    p2_half(1, 1, 0); p2_half(1, 1, 1)
                    p2_half(0, 1, 0); p3_store(0, 1, 0)
                    p3_store(1, 1, 0)
                    p2_half(0, 1, 1); p3_store(0, 1, 1)
                    p3_store(1, 1, 1)
                else:
                    for k, j in unit_order:
                        for hhalf in range(2):
                            p2_half(k, j, hhalf)
                            p3_store(k, j, hhalf)

    nc.compile()
    return nc


_NC_CACHE = None


def _get_module():
    global _NC_CACHE
    if _NC_CACHE is None:
        _NC_CACHE = build_module()
    return _NC_CACHE


def make_in_maps(inputs):
    reps = {k: np.ascontiguousarray(v) for k, v in inputs.items()
            if k not in ("x", "residual")}
    in_maps = []
    for core in range(N_CORES):
        bs = slice(core * NLOC, (core + 1) * NLOC)
        m = {"x": np.ascontiguousarray(inputs["x"][bs]),
             "residual": np.ascontiguousarray(inputs["residual"][bs])}
        m.update(reps)
        in_maps.append(m)
    return in_maps


def run_spmd(nc, in_maps):
    res = run_bass_kernel_spmd(nc, in_maps, core_ids=list(range(N_CORES)))
    return np.concatenate([res.results[c]["out"] for c in range(N_CORES)], axis=0)


def kernel(**inputs):
    inputs = {k: np.asarray(v) for k, v in inputs.items()}
    nc = _get_module()
    return run_spmd(nc, make_in_maps(inputs))


# revision 10
# speedup vs baseline: 1.0085x; 1.0085x over previous
"""CoordAtt Trainium2 Bass kernel (v3).

Reference computation (per batch n, c=256, h=w=64, mip=8):
    xs   = x + residual                      (bilinear resize of residual at
                                              identical shape is the identity)
    y    = concat(mean_w(xs), mean_h(xs))    -> [mip, h+w]
    y    = hswish(BN(w1 @ y + b1))           -> [mip, h+w]
    a_h  = sigmoid(w2 @ y[:, :h] + b2)       -> [c, h]
    a_w  = sigmoid(w3 @ y[:, h:] + b3)       -> [c, w]
    out  = 2*xs*a_h*a_w + 2*residual*(1 - a_h*a_w)
         = (2*a_h)*a_w*x + 2*residual        (algebraically identical)

Design (8 cores, data-parallel over batch n: 2 batches/core). The kernel is
HBM-bound: 24 MiB/core mandatory traffic at 360 B/ns ~= 70 us, so everything
else is scheduled to hide under the DMA stream:

  * conv-before-pool in float32r (fp32 at full PE rate, no bf16 casts):
    y = w1'^T @ x + w1'^T @ res via PSUM accumulation, where w1' has the
    folded BN scale (gamma/sqrt(var+eps)/W) pre-multiplied during on-chip
    weight prep (transpose matmul against diag(scale)).  The conv output is
    PARTITION-STACKED: the stationary for quarter q is w1'^T replicated 4x
    into 32-column block q of a [c, 64] tile (zeros elsewhere), so PSUM
    accumulation of the two quarters lands each h-quarter in its own
    32-partition block of a [64, 1024] psum tile per half.  The
    directional pools then reduce a free dim of 1024 instead of 4096 (4x
    less DVE time).  Identity column-slice matmuls (built on-chip via
    affine_select) gather pooled values back to [mip, h+w] layout.
  * hswish folds to 3 ops: u = relu(y + bias+3); v = (u-3) * min(u,6)/6.
  * weights are DMA'd contiguously, interleaved into the sync-queue load
    stream so their HWDGE descriptor slots never starve the DMA engines.
  * finals: t = x*ah (f32->bf16), t *= 2*aw (bf16: DVE 2x mode), out =
    2r + t, stored at half-tile granularity.  GpSimd has no fused
    scalar_tensor_tensor on hardware, so its units use Act (o=2r) + a
    GpSimd add; engine assignment and emission order are hand-tuned so
    batch 1's output stream meets the DMA slots right after its loads.
  * hardware ISA quirks found the hard way: fp32r matmuls may only write
    dst partition base 0 (hence the masked stationaries), and any matmul
    operand at partition base 32 kills the device (hence identity
    column-slice gathers instead of partition-sliced stationaries).
"""

import numpy as np

import concourse.bacc as bacc
import concourse.mybir as mybir
from concourse.tile import TileContext
from concourse.bass_utils import run_bass_kernel_spmd

F32 = mybir.dt.float32
F32R = mybir.dt.float32r
BF16 = mybir.dt.bfloat16
Alu = mybir.AluOpType
Act = mybir.ActivationFunctionType
AX = mybir.AxisListType

N_CORES = 8
N, C, H, W = 16, 256, 64, 64
NLOC = N // N_CORES           # batches per core
MIP = 8
EPS = 1e-5
HW = H * W                    # 4096 flat (h w) columns per (batch, chunk)
NCHUNK = C // 128             # channel chunks (2)
HCOL = HW // 2                # 2048 columns per half tile
QH = H // 4                   # h rows per quarter (16)
QCOL = QH * W                 # columns per quarter (1024)


def build_module():
    nc = bacc.Bacc("TRN2", target_bir_lowering=False)

    x_d = nc.dram_tensor("x", (NLOC, C, H, W), F32, kind="ExternalInput")
    r_d = nc.dram_tensor("residual", (NLOC, C, H, W), F32, kind="ExternalInput")
    w1_d = nc.dram_tensor("w1", (MIP, C), F32, kind="ExternalInput")
    b1_d = nc.dram_tensor("b1", (MIP,), F32, kind="ExternalInput")
    gamma_d = nc.dram_tensor("bn_gamma", (MIP,), F32, kind="ExternalInput")
    beta_d = nc.dram_tensor("bn_beta", (MIP,), F32, kind="ExternalInput")
    mean_d = nc.dram_tensor("bn_mean", (MIP,), F32, kind="ExternalInput")
    var_d = nc.dram_tensor("bn_var", (MIP,), F32, kind="ExternalInput")
    w2_d = nc.dram_tensor("w2", (C, MIP), F32, kind="ExternalInput")
    b2_d = nc.dram_tensor("b2", (C,), F32, kind="ExternalInput")
    w3_d = nc.dram_tensor("w3", (C, MIP), F32, kind="ExternalInput")
    b3_d = nc.dram_tensor("b3", (C,), F32, kind="ExternalInput")
    out_d = nc.dram_tensor("out", (NLOC, C, H, W), F32, kind="ExternalOutput")

    with TileContext(nc) as tc:
        with (
            tc.tile_pool(name="big", bufs=1) as big,
            tc.tile_pool(name="tbf", bufs=1) as tbf,
            tc.tile_pool(name="small", bufs=1) as small,
            tc.tile_pool(name="work", bufs=2) as work,
            tc.tile_pool(name="psum_y", bufs=1, space="PSUM") as psum_y,
            tc.tile_pool(name="psum_mlp", bufs=1, space="PSUM") as psum_mlp,
            tc.tile_pool(name="psum_att", bufs=1, space="PSUM") as psum_att,
        ):
            # ---- input loads + weight DMAs, all on the sync queue so HWDGE
            # grants follow this order and the DMA engines never starve;
            # tiny weight transfers slot between the 1 MiB input tiles.
            xt = {}
            rt = {}
            w_tiles = {}

            def load(b, k, j):
                js = slice(j * HCOL, (j + 1) * HCOL)
                cs = slice(k * 128, (k + 1) * 128)
                t = big.tile([128, HCOL], F32, name=f"x_{b}_{k}_{j}", tag=f"x{b}{k}{j}")
                nc.sync.dma_start(t[:].bitcast(F32R),
                                  x_d[b, cs].rearrange("c h w -> c (h w)")[:, js].bitcast(F32R))
                xt[b, k, j] = t
                t = big.tile([128, HCOL], F32, name=f"r_{b}_{k}_{j}", tag=f"r{b}{k}{j}")
                nc.sync.dma_start(t[:].bitcast(F32R),
                                  r_d[b, cs].rearrange("c h w -> c (h w)")[:, js].bitcast(F32R))
                rt[b, k, j] = t

            load(0, 0, 0)
            w1f = small.tile([MIP, C], F32, tag="w1f")
            nc.sync.dma_start(w1f[:], w1_d[:, :])
            bn_in = small.tile([MIP, 5], F32, tag="bn_in")
            for i, d in enumerate((var_d, gamma_d, beta_d, mean_d, b1_d)):
                nc.sync.dma_start(bn_in[:, i:i + 1], d[:].unsqueeze(1))
            load(0, 1, 0)
            w2f = small.tile([128, 2 * MIP], F32, tag="w2f")
            nc.sync.dma_start(w2f[:].rearrange("p (k o) -> p k o", k=2),
                              w2_d.rearrange("(k p) o -> p k o", p=128))
            w3f = small.tile([128, 2 * MIP], F32, tag="w3f")
            nc.sync.dma_start(w3f[:].rearrange("p (k o) -> p k o", k=2),
                              w3_d.rearrange("(k p) o -> p k o", p=128))
            b2t = small.tile([128, NCHUNK], F32, tag="b2t")
            nc.sync.dma_start(b2t[:], b2_d.rearrange("(k p) -> p k", p=128))
            b3t = small.tile([128, NCHUNK], F32, tag="b3t")
            nc.sync.dma_start(b3t[:], b3_d.rearrange("(k p) -> p k", p=128))
            for b in range(NLOC):
                for j in range(2):
                    for k in range(NCHUNK):
                        if (b, k, j) not in xt:
                            load(b, k, j)

            # ---- identity / selection matrices (on-chip constants) ----
            ones = small.tile([128, 128], F32, tag="ones")
            nc.vector.memset(ones[:], 1.0)
            ident = small.tile([128, 128], F32, tag="ident")
            nc.gpsimd.affine_select(ident[:], ones[:], [[-1, 128]], Alu.is_equal, 0.0,
                                    base=0, channel_multiplier=1)
            # warm the Act function tables (copy/relu/sqrt set + sigmoid set)
            # during the idle startup so they don't stall the first real ops
            warm = small.tile([MIP, 2], F32, tag="warm")
            nc.vector.memset(warm[:, 0:1], 0.0)
            nc.scalar.copy(warm[:, 1:2], warm[:, 0:1])
            nc.scalar.activation(warm[:, 1:2], warm[:, 0:1], Act.Sigmoid)
            # S32[p, m] = 1 iff p mod 32 == m (m < 8): pool-gather stationary
            s32 = small.tile([128, MIP], F32, tag="s32")
            nc.gpsimd.tensor_tensor(s32[:], ident[:, 0:MIP], ident[:, 32:32 + MIP], Alu.add)
            nc.gpsimd.tensor_tensor(s32[:], s32[:], ident[:, 64:64 + MIP], Alu.add)
            nc.gpsimd.tensor_tensor(s32[:], s32[:], ident[:, 96:96 + MIP], Alu.add)

            # ---- BN fold ----
            # scale_p = gamma/sqrt(var+eps)/W (folded into w1'), bias3_p =
            # (b1-mean)*inv + beta + 3 (the +3 pre-biases relu6(y+3))
            consts = small.tile([MIP, 2], F32, tag="consts")
            nc.vector.memset(consts[:, 0:1], EPS)
            var_c = bn_in[:, 0:1]
            gamma_c = bn_in[:, 1:2]
            beta_c = bn_in[:, 2:3]
            mean_c = bn_in[:, 3:4]
            b1_c = bn_in[:, 4:5]
            bn_t = small.tile([MIP, 4], F32, tag="bn_t")
            sv = bn_t[:, 0:1]
            inv = bn_t[:, 1:2]
            scale_p = bn_t[:, 2:3]
            bias3_p = bn_t[:, 3:4]
            nc.scalar.activation(sv, var_c, Act.Sqrt, bias=consts[:, 0:1], scale=1.0)
            nc.vector.reciprocal(inv, sv)
            nc.vector.tensor_tensor(inv, inv, gamma_c, Alu.mult)
            nc.vector.tensor_scalar_mul(scale_p, inv, 1.0 / W)
            nc.vector.tensor_tensor(bias3_p, b1_c, mean_c, Alu.subtract)
            nc.vector.scalar_tensor_tensor(bias3_p, bias3_p, inv, beta_c, Alu.mult, Alu.add)
            nc.vector.tensor_scalar_add(bias3_p, bias3_p, 3.0)

            zeros = small.tile([128, 1], F32, tag="zeros")
            nc.vector.memset(zeros[:], 0.0)

            # ---- weight prep on PE (plain matmuls against identity/diag) ----
            # ds = diag(scale_p): transpose-matmul against it folds BN scale
            ds = small.tile([MIP, MIP], F32, tag="ds")
            nc.vector.tensor_tensor(ds[:], ident[0:MIP, 0:MIP],
                                    scale_p.broadcast_to((MIP, MIP)), Alu.mult)
            # w1m[k][q]: [c128, 64] masked stationary: (w1^T . diag(scale))
            # chunk k replicated 4x in 32-column block q, zeros in the other
            # block.  PSUM accumulation of the two quarters then yields the
            # partition-stacked conv output with dst partition base 0 (the
            # only base the fp32r matmul ISA allows).
            w1m = []
            for k in range(NCHUNK):
                ps = psum_att.tile([128, MIP], F32, name=f"w1tp{k}", tag=f"att{k}")
                nc.tensor.matmul(ps[:], w1f[:, k * 128:(k + 1) * 128], ds[:],
                                 start=True, stop=True)
                per_q = []
                for q in range(2):
                    t = small.tile([128, 64], F32R, tag=f"w1m_{k}_{q}")
                    nc.scalar.copy(
                        t[:, 32 * (1 - q):32 * (2 - q)],
                        zeros[:].broadcast_to((128, 32)))
                    nc.scalar.copy(
                        t[:, 32 * q:32 * (q + 1)].rearrange("p (a m) -> p a m", a=4),
                        ps[:].unsqueeze(1).broadcast_to((128, 4, MIP)))
                    per_q.append(t)
                w1m.append(per_q)
            # w2t/w3t: [mip, C] via transpose-matmul against I128
            w2t = small.tile([MIP, C], F32, tag="w2t")
            w3t = small.tile([MIP, C], F32, tag="w3t")
            for wt, wf in ((w2t, w2f), (w3t, w3f)):
                for k in range(NCHUNK):
                    ps = psum_mlp.tile([MIP, 128], F32, name=f"wp_{id(wf)}_{k}", tag=f"mlp{k}")
                    nc.tensor.matmul(ps[:], wf[:, k * MIP:(k + 1) * MIP], ident[:, :],
                                     start=True, stop=True)
                    nc.scalar.copy(wt[:, k * 128:(k + 1) * 128], ps[:])

            # ---- per-batch pipeline ----
            # engine assignment per batch for the finals: batch 0 has slack,
            # so its p3s all go to GpSimd (keeping DVE free for batch 1's
            # pool reduces); batch 1's tail is latency-critical, so DVE takes
            # most p3s and the last unit's p1 is emitted late in the finals.
            LAST = NLOC - 1
            for b in range(NLOC):
                p1_eng = {(0, 0): "D", (1, 0): "P", (0, 1): "D", (1, 1): "P"}
                p3_eng = {(0, 0): "D", (1, 0): "P", (0, 1): "D", (1, 1): "P"}
                if b == LAST:
                    p1_late = {(0, 1)}          # emitted inside the finals
                else:
                    p1_late = set()
                    # GpSimd units stored first so batch 1's o-tile reuse
                    # (WAR on these stores) clears early
                    unit_order = [(1, 0), (1, 1), (0, 0), (0, 1)]
                vy = work.tile([MIP, 2 * H], F32, name=f"v_{b}", tag="v")
                mlp_ps = psum_mlp.tile([MIP, 2 * H], F32, name=f"mlp_{b}", tag=f"mlp{b % 2}")
                ah = {}
                aw_bf = {}
                tbf_t = {}
                for k in range(NCHUNK):
                    ah[k] = work.tile([128, H], F32, name=f"ah_{b}_{k}", tag=f"ah_{k}")
                    aw_bf[k] = work.tile([128, W], BF16, name=f"awbf_{b}_{k}", tag=f"awbf_{k}")

                def emit_p1(k, j):
                    t = tbf.tile([128, HCOL], BF16, name=f"t_{b}_{k}_{j}", tag=f"t{k}{j}")
                    tbf_t[k, j] = t
                    xs_ = xt[b, k, j][:].rearrange("p (h w) -> p h w", h=32)
                    ahb = ah[k][:, 32 * j:32 * j + 32].unsqueeze(2).broadcast_to((128, 32, W))
                    tv = t[:].rearrange("p (h w) -> p h w", h=32)
                    eng = nc.vector if p1_eng[k, j] == "D" else nc.gpsimd
                    eng.tensor_tensor(tv, xs_, ahb, Alu.mult)

                def emit_mlp(ps_slice, v_slice, u, m6):
                    # u = relu(y + bias + 3); v = (u-3)*min(u,6)/6 == hswish(BN(y))
                    nc.scalar.activation(u[:], ps_slice, Act.Relu, bias=bias3_p, scale=1.0)
                    nc.vector.tensor_scalar(m6[:], u[:], 6.0, 1.0 / 6.0, Alu.min, Alu.mult)
                    nc.vector.scalar_tensor_tensor(v_slice, u[:], -3.0, m6[:],
                                                   Alu.add, Alu.mult)

                for j in range(2):
                    # conv for half j: psum tile [64, 1024]; quarter q of this
                    # half at partition base 32q; 512-col slices accumulate
                    # x0, r0, x1, r1 (x+res for free).
                    y_ps = psum_y.tile([64, QCOL], F32, name=f"y_{b}_{j}", tag=f"y{j}")
                    srcs = [(0, xt[b, 0, j]), (0, rt[b, 0, j]),
                            (1, xt[b, 1, j]), (1, rt[b, 1, j])]
                    for s in range(2):
                        accum = [(q, k, src) for (k, src) in srcs for q in range(2)]
                        for i, (q, k, src) in enumerate(accum):
                            col = q * QCOL + s * 512
                            nc.tensor.matmul(
                                y_ps[0:64, s * 512:(s + 1) * 512],
                                w1m[k][q][:],
                                src[:, col:col + 512].bitcast(F32R),
                                start=(i == 0), stop=(i == len(accum) - 1),
                            )
                    # directional pools (DVE): free dim is just 1024.  The
                    # a_w path is the batch's critical path, so the column
                    # pool goes first.
                    ywp_s = work.tile([64, W], F32, name=f"ywp_{b}_{j}", tag=f"ywp{j}")
                    nc.vector.reduce_sum(
                        ywp_s[:], y_ps[:].rearrange("p (h w) -> p w h", h=QH), axis=AX.X)
                    # yw gather-sum accumulates across halves
                    nc.tensor.matmul(mlp_ps[:, H:2 * H], s32[0:64, :], ywp_s[:],
                                     start=(j == 0), stop=(j == 1))
                    yh_s = work.tile([64, QH], F32, name=f"yh_{b}_{j}", tag=f"yh{j}")
                    nc.vector.reduce_sum(
                        yh_s[:], y_ps[:].rearrange("p (h w) -> p h w", h=QH), axis=AX.X)
                    if j == 1:
                        # a_w chain as soon as the last column pool lands
                        ws_ = slice(H, 2 * H)
                        uw = work.tile([MIP, W], F32, name=f"uw_{b}", tag="uw")
                        m6w = work.tile([MIP, W], F32, name=f"m6w_{b}", tag="m6w")
                        emit_mlp(mlp_ps[:, ws_], vy[:, ws_], uw, m6w)
                        for k in range(NCHUNK):
                            cs = slice(k * 128, (k + 1) * 128)
                            awp = psum_att.tile([128, W], F32, name=f"awp_{b}_{k}", tag=f"att{k}")
                            nc.tensor.matmul(awp[:], w3t[:, cs], vy[:, ws_],
                                             start=True, stop=True)
                            nc.scalar.activation(aw_bf[k][:], awp[:], Act.Sigmoid,
                                                 bias=b3t[:, k:k + 1], scale=1.0)
                            nc.scalar.mul(aw_bf[k][:], aw_bf[k][:], 2.0)
                    # gather yh quarters -> mlp psum cols [32j, 32j+32)
                    # gather quarter q's rows via an identity column-slice
                    # stationary at base 0 (partition-offset operands are
                    # rejected by the hardware ISA)
                    for q in range(2):
                        nc.tensor.matmul(
                            mlp_ps[:, 32 * j + QH * q: 32 * j + QH * (q + 1)],
                            ident[0:64, 32 * q:32 * q + MIP],
                            yh_s[:, :],
                            start=True, stop=True)
                    # mlp + a_h for this half
                    hs = slice(32 * j, 32 * j + 32)
                    u = work.tile([MIP, 32], F32, name=f"u_{b}_{j}", tag=f"u{j}")
                    m6 = work.tile([MIP, 32], F32, name=f"m6_{b}_{j}", tag=f"m6{j}")
                    emit_mlp(mlp_ps[:, hs], vy[:, hs], u, m6)
                    for k in range(NCHUNK):
                        cs = slice(k * 128, (k + 1) * 128)
                        ahp = psum_att.tile([128, 32], F32, name=f"ahp_{b}_{j}_{k}",
                                            tag=f"att{(2 * j + k) % 2}")
                        nc.tensor.matmul(ahp[:], w2t[:, cs], vy[:, hs], start=True, stop=True)
                        nc.scalar.activation(ah[k][:, hs], ahp[:], Act.Sigmoid,
                                             bias=b2t[:, k:k + 1], scale=1.0)
                    # p1: t = (2x) * ah (f32 -> bf16)
                    for k in range(NCHUNK):
                        if (k, j) not in p1_late:
                            emit_p1(k, j)

                # finals: p2 on DVE (bf16 2x) and p3 + store, at half-tile
                # granularity so the output stream starts right after a_w
                # lands.  The critical batch uses a hand-interleaved order so
                # DVE and GpSimd finish together and every store meets its
                # DMA slot.
                ot = {}
                for j in range(2):
                    for k in range(NCHUNK):
                        # the last batch's outputs reuse batch 0's x buffers
                        # (dead after batch 0's conv+p1) to avoid WAR stalls
                        # against batch 0's outgoing stores
                        tag = f"x0{k}{j}" if b == LAST else f"o{k}{j}"
                        ot[k, j] = big.tile([128, HCOL], F32, name=f"o_{b}_{k}_{j}",
                                            tag=tag)

                def p2_half(k, j, hhalf):
                    t = tbf_t[k, j]
                    sl = slice(hhalf * 1024, (hhalf + 1) * 1024)
                    tv = t[:, sl].rearrange("p (h w) -> p h w", h=16)
                    awb = aw_bf[k][:].unsqueeze(1).broadcast_to((128, 16, W))
                    nc.vector.tensor_tensor(tv, tv, awb, Alu.mult)

                def pre2r(k, j):
                    # o = 2r for GpSimd p3 units (Act has slack; GpSimd lacks
                    # a fused scalar_tensor_tensor on hardware)
                    nc.scalar.mul(ot[k, j][:], rt[b, k, j][:], 2.0)

                def p3_store(k, j, hhalf):
                    t = tbf_t[k, j]
                    r = rt[b, k, j]
                    o = ot[k, j]
                    sl = slice(hhalf * 1024, (hhalf + 1) * 1024)
                    if p3_eng[k, j] == "D":
                        nc.vector.scalar_tensor_tensor(
                            o[:, sl], r[:, sl], 2.0, t[:, sl], Alu.mult, Alu.add)
                    else:
                        nc.gpsimd.tensor_tensor(o[:, sl], o[:, sl], t[:, sl], Alu.add)
                    od = out_d[b, k * 128:(k + 1) * 128].rearrange("c h w -> c (h w)")
                    nc.sync.dma_start(
                        od[:, j * HCOL + sl.start: j * HCOL + sl.stop], o[:, sl])

                for (kk, jj), e in p3_eng.items():
                    if e != "D":
                        pre2r(kk, jj)
                if b == LAST:
                    p2_half(0, 0, 0); p3_store(0, 0, 0)
                    p2_half(0, 0, 1); p3_store(0, 0, 1)
                    p2_half(1, 0, 0); p2_half(1, 0, 1)
                    p3_store(1, 0, 0); p3_store(1, 0, 1)
                    emit_p1(0, 1)
                    p2_half(1, 1, 0); p2_half(1, 1, 1)
                    p2_half(0, 1, 0); p3_store(0, 1, 0)
                    p3_store(1, 1, 0)
                    p2_half(0, 1, 1); p3_store(0, 1, 1)
                    p3_store(1, 1, 1)
                else:
                    for k, j in unit_order:
                        for hhalf in range(2):
                            p2_half(k, j, hhalf)
                            p3_store(k, j, hhalf)

    nc.compile()
    return nc


_NC_CACHE = None


def _get_module():
    global _NC_CACHE
    if _NC_CACHE is None:
        _NC_CACHE = build_module()
    return _NC_CACHE


def make_in_maps(inputs):
    reps = {k: np.ascontiguousarray(v) for k, v in inputs.items()
            if k not in ("x", "residual")}
    in_maps = []
    for core in range(N_CORES):
        bs = slice(core * NLOC, (core + 1) * NLOC)
        m = {"x": np.ascontiguousarray(inputs["x"][bs]),
             "residual": np.ascontiguousarray(inputs["residual"][bs])}
        m.update(reps)
        in_maps.append(m)
    return in_maps


def run_spmd(nc, in_maps):
    res = run_bass_kernel_spmd(nc, in_maps, core_ids=list(range(N_CORES)))
    return np.concatenate([res.results[c]["out"] for c in range(N_CORES)], axis=0)


def kernel(**inputs):
    inputs = {k: np.asarray(v) for k, v in inputs.items()}
    nc = _get_module()
    return run_spmd(nc, make_in_maps(inputs))


# revision 11
# speedup vs baseline: 1.0106x; 1.0021x over previous
"""CoordAtt Trainium2 Bass kernel (v3).

Reference computation (per batch n, c=256, h=w=64, mip=8):
    xs   = x + residual                      (bilinear resize of residual at
                                              identical shape is the identity)
    y    = concat(mean_w(xs), mean_h(xs))    -> [mip, h+w]
    y    = hswish(BN(w1 @ y + b1))           -> [mip, h+w]
    a_h  = sigmoid(w2 @ y[:, :h] + b2)       -> [c, h]
    a_w  = sigmoid(w3 @ y[:, h:] + b3)       -> [c, w]
    out  = 2*xs*a_h*a_w + 2*residual*(1 - a_h*a_w)
         = (2*a_h)*a_w*x + 2*residual        (algebraically identical)

Design (8 cores, data-parallel over batch n: 2 batches/core). The kernel is
HBM-bound: 24 MiB/core mandatory traffic at 360 B/ns ~= 70 us, so everything
else is scheduled to hide under the DMA stream:

  * conv-before-pool in float32r (fp32 at full PE rate, no bf16 casts):
    y = w1'^T @ x + w1'^T @ res via PSUM accumulation, where w1' has the
    folded BN scale (gamma/sqrt(var+eps)/W) pre-multiplied during on-chip
    weight prep (transpose matmul against diag(scale)).  The conv output is
    PARTITION-STACKED: the stationary for quarter q is w1'^T replicated 4x
    into 32-column block q of a [c, 64] tile (zeros elsewhere), so PSUM
    accumulation of the two quarters lands each h-quarter in its own
    32-partition block of a [64, 1024] psum tile per half.  The
    directional pools then reduce a free dim of 1024 instead of 4096 (4x
    less DVE time).  Identity column-slice matmuls (built on-chip via
    affine_select) gather pooled values back to [mip, h+w] layout.
  * hswish folds to 3 ops: u = relu(y + bias+3); v = (u-3) * min(u,6)/6.
  * weights are DMA'd contiguously, interleaved into the sync-queue load
    stream so their HWDGE descriptor slots never starve the DMA engines.
  * finals: t = x*ah (f32->bf16), t *= 2*aw (bf16: DVE 2x mode), out =
    2r + t, stored at half-tile granularity.  GpSimd has no fused
    scalar_tensor_tensor on hardware, so its units use Act (o=2r) + a
    GpSimd add; engine assignment and emission order are hand-tuned so
    batch 1's output stream meets the DMA slots right after its loads.
  * hardware ISA quirks found the hard way: fp32r matmuls may only write
    dst partition base 0 (hence the masked stationaries), and any matmul
    operand at partition base 32 kills the device (hence identity
    column-slice gathers instead of partition-sliced stationaries).
"""

import numpy as np

import concourse.bacc as bacc
import concourse.mybir as mybir
from concourse.tile import TileContext
from concourse.bass_utils import run_bass_kernel_spmd

F32 = mybir.dt.float32
F32R = mybir.dt.float32r
BF16 = mybir.dt.bfloat16
Alu = mybir.AluOpType
Act = mybir.ActivationFunctionType
AX = mybir.AxisListType

N_CORES = 8
N, C, H, W = 16, 256, 64, 64
NLOC = N // N_CORES           # batches per core
MIP = 8
EPS = 1e-5
HW = H * W                    # 4096 flat (h w) columns per (batch, chunk)
NCHUNK = C // 128             # channel chunks (2)
HCOL = HW // 2                # 2048 columns per half tile
QH = H // 4                   # h rows per quarter (16)
QCOL = QH * W                 # columns per quarter (1024)


def build_module():
    nc = bacc.Bacc("TRN2", target_bir_lowering=False)

    x_d = nc.dram_tensor("x", (NLOC, C, H, W), F32, kind="ExternalInput")
    r_d = nc.dram_tensor("residual", (NLOC, C, H, W), F32, kind="ExternalInput")
    w1_d = nc.dram_tensor("w1", (MIP, C), F32, kind="ExternalInput")
    b1_d = nc.dram_tensor("b1", (MIP,), F32, kind="ExternalInput")
    gamma_d = nc.dram_tensor("bn_gamma", (MIP,), F32, kind="ExternalInput")
    beta_d = nc.dram_tensor("bn_beta", (MIP,), F32, kind="ExternalInput")
    mean_d = nc.dram_tensor("bn_mean", (MIP,), F32, kind="ExternalInput")
    var_d = nc.dram_tensor("bn_var", (MIP,), F32, kind="ExternalInput")
    w2_d = nc.dram_tensor("w2", (C, MIP), F32, kind="ExternalInput")
    b2_d = nc.dram_tensor("b2", (C,), F32, kind="ExternalInput")
    w3_d = nc.dram_tensor("w3", (C, MIP), F32, kind="ExternalInput")
    b3_d = nc.dram_tensor("b3", (C,), F32, kind="ExternalInput")
    out_d = nc.dram_tensor("out", (NLOC, C, H, W), F32, kind="ExternalOutput")

    with TileContext(nc) as tc:
        with (
            tc.tile_pool(name="big", bufs=1) as big,
            tc.tile_pool(name="tbf", bufs=1) as tbf,
            tc.tile_pool(name="small", bufs=1) as small,
            tc.tile_pool(name="work", bufs=2) as work,
            tc.tile_pool(name="psum_y", bufs=1, space="PSUM") as psum_y,
            tc.tile_pool(name="psum_mlp", bufs=1, space="PSUM") as psum_mlp,
            tc.tile_pool(name="psum_att", bufs=1, space="PSUM") as psum_att,
        ):
            # ---- input loads + weight DMAs, all on the sync queue so HWDGE
            # grants follow this order and the DMA engines never starve;
            # tiny weight transfers slot between the 1 MiB input tiles.
            xt = {}
            rt = {}
            w_tiles = {}

            def load(b, k, j):
                js = slice(j * HCOL, (j + 1) * HCOL)
                cs = slice(k * 128, (k + 1) * 128)
                t = big.tile([128, HCOL], F32, name=f"x_{b}_{k}_{j}", tag=f"x{b}{k}{j}")
                nc.sync.dma_start(t[:].bitcast(F32R),
                                  x_d[b, cs].rearrange("c h w -> c (h w)")[:, js].bitcast(F32R))
                xt[b, k, j] = t
                t = big.tile([128, HCOL], F32, name=f"r_{b}_{k}_{j}", tag=f"r{b}{k}{j}")
                nc.sync.dma_start(t[:].bitcast(F32R),
                                  r_d[b, cs].rearrange("c h w -> c (h w)")[:, js].bitcast(F32R))
                rt[b, k, j] = t

            load(0, 0, 0)
            w1f = small.tile([MIP, C], F32, tag="w1f")
            nc.sync.dma_start(w1f[:], w1_d[:, :])
            bn_in = small.tile([MIP, 5], F32, tag="bn_in")
            for i, d in enumerate((var_d, gamma_d, beta_d, mean_d, b1_d)):
                nc.sync.dma_start(bn_in[:, i:i + 1], d[:].unsqueeze(1))
            load(0, 1, 0)
            w2f = small.tile([128, 2 * MIP], F32, tag="w2f")
            nc.sync.dma_start(w2f[:].rearrange("p (k o) -> p k o", k=2),
                              w2_d.rearrange("(k p) o -> p k o", p=128))
            w3f = small.tile([128, 2 * MIP], F32, tag="w3f")
            nc.sync.dma_start(w3f[:].rearrange("p (k o) -> p k o", k=2),
                              w3_d.rearrange("(k p) o -> p k o", p=128))
            b2t = small.tile([128, NCHUNK], F32, tag="b2t")
            nc.sync.dma_start(b2t[:], b2_d.rearrange("(k p) -> p k", p=128))
            b3t = small.tile([128, NCHUNK], F32, tag="b3t")
            nc.sync.dma_start(b3t[:], b3_d.rearrange("(k p) -> p k", p=128))
            for b in range(NLOC):
                for j in range(2):
                    for k in range(NCHUNK):
                        if (b, k, j) not in xt:
                            load(b, k, j)

            # ---- identity / selection matrices (on-chip constants) ----
            ones = small.tile([128, 128], F32, tag="ones")
            nc.vector.memset(ones[:], 1.0)
            ident = small.tile([128, 128], F32, tag="ident")
            nc.gpsimd.affine_select(ident[:], ones[:], [[-1, 128]], Alu.is_equal, 0.0,
                                    base=0, channel_multiplier=1)
            # warm the Act function tables (copy/relu/sqrt set + sigmoid set)
            # during the idle startup so they don't stall the first real ops
            warm = small.tile([MIP, 2], F32, tag="warm")
            nc.vector.memset(warm[:, 0:1], 0.0)
            nc.scalar.copy(warm[:, 1:2], warm[:, 0:1])
            nc.scalar.activation(warm[:, 1:2], warm[:, 0:1], Act.Sigmoid)
            # S32[p, m] = 1 iff p mod 32 == m (m < 8): pool-gather stationary
            s32 = small.tile([128, MIP], F32, tag="s32")
            nc.gpsimd.tensor_tensor(s32[:], ident[:, 0:MIP], ident[:, 32:32 + MIP], Alu.add)
            nc.gpsimd.tensor_tensor(s32[:], s32[:], ident[:, 64:64 + MIP], Alu.add)
            nc.gpsimd.tensor_tensor(s32[:], s32[:], ident[:, 96:96 + MIP], Alu.add)

            # ---- BN fold ----
            # scale_p = gamma/sqrt(var+eps)/W (folded into w1'), bias3_p =
            # (b1-mean)*inv + beta + 3 (the +3 pre-biases relu6(y+3))
            consts = small.tile([MIP, 2], F32, tag="consts")
            nc.vector.memset(consts[:, 0:1], EPS)
            var_c = bn_in[:, 0:1]
            gamma_c = bn_in[:, 1:2]
            beta_c = bn_in[:, 2:3]
            mean_c = bn_in[:, 3:4]
            b1_c = bn_in[:, 4:5]
            bn_t = small.tile([MIP, 4], F32, tag="bn_t")
            sv = bn_t[:, 0:1]
            inv = bn_t[:, 1:2]
            scale_p = bn_t[:, 2:3]
            bias3_p = bn_t[:, 3:4]
            nc.scalar.activation(sv, var_c, Act.Sqrt, bias=consts[:, 0:1], scale=1.0)
            nc.vector.reciprocal(inv, sv)
            nc.vector.tensor_tensor(inv, inv, gamma_c, Alu.mult)
            nc.vector.tensor_scalar_mul(scale_p, inv, 1.0 / W)
            nc.vector.tensor_tensor(bias3_p, b1_c, mean_c, Alu.subtract)
            nc.vector.scalar_tensor_tensor(bias3_p, bias3_p, inv, beta_c, Alu.mult, Alu.add)
            nc.vector.tensor_scalar_add(bias3_p, bias3_p, 3.0)

            zeros = small.tile([128, 1], F32, tag="zeros")
            nc.vector.memset(zeros[:], 0.0)

            # ---- weight prep on PE (plain matmuls against identity/diag) ----
            # ds = diag(scale_p): transpose-matmul against it folds BN scale
            ds = small.tile([MIP, MIP], F32, tag="ds")
            nc.vector.tensor_tensor(ds[:], ident[0:MIP, 0:MIP],
                                    scale_p.broadcast_to((MIP, MIP)), Alu.mult)
            # w1m[k][q]: [c128, 64] masked stationary: (w1^T . diag(scale))
            # chunk k replicated 4x in 32-column block q, zeros in the other
            # block.  PSUM accumulation of the two quarters then yields the
            # partition-stacked conv output with dst partition base 0 (the
            # only base the fp32r matmul ISA allows).
            w1m = []
            for k in range(NCHUNK):
                ps = psum_att.tile([128, MIP], F32, name=f"w1tp{k}", tag=f"att{k}")
                nc.tensor.matmul(ps[:], w1f[:, k * 128:(k + 1) * 128], ds[:],
                                 start=True, stop=True)
                per_q = []
                for q in range(2):
                    t = small.tile([128, 64], F32R, tag=f"w1m_{k}_{q}")
                    nc.scalar.copy(
                        t[:, 32 * (1 - q):32 * (2 - q)],
                        zeros[:].broadcast_to((128, 32)))
                    nc.scalar.copy(
                        t[:, 32 * q:32 * (q + 1)].rearrange("p (a m) -> p a m", a=4),
                        ps[:].unsqueeze(1).broadcast_to((128, 4, MIP)))
                    per_q.append(t)
                w1m.append(per_q)
            # w2t/w3t: [mip, C] via transpose-matmul against I128
            w2t = small.tile([MIP, C], F32, tag="w2t")
            w3t = small.tile([MIP, C], F32, tag="w3t")
            for wt, wf in ((w2t, w2f), (w3t, w3f)):
                for k in range(NCHUNK):
                    ps = psum_mlp.tile([MIP, 128], F32, name=f"wp_{id(wf)}_{k}", tag=f"mlp{k}")
                    nc.tensor.matmul(ps[:], wf[:, k * MIP:(k + 1) * MIP], ident[:, :],
                                     start=True, stop=True)
                    nc.scalar.copy(wt[:, k * 128:(k + 1) * 128], ps[:])

            # ---- per-batch pipeline ----
            # engine assignment per batch for the finals: batch 0 has slack,
            # so its p3s all go to GpSimd (keeping DVE free for batch 1's
            # pool reduces); batch 1's tail is latency-critical, so DVE takes
            # most p3s and the last unit's p1 is emitted late in the finals.
            LAST = NLOC - 1
            for b in range(NLOC):
                p1_eng = {(0, 0): "D", (1, 0): "P", (0, 1): "D", (1, 1): "P"}
                if b == LAST:
                    # DVE owns every p3 except u11's so the first four output
                    # stores stream gapless from the fused STT path
                    p3_eng = {(0, 0): "D", (1, 0): "D", (0, 1): "D", (1, 1): "P"}
                    p1_late = {(0, 1)}          # emitted inside the finals
                else:
                    p3_eng = {(0, 0): "D", (1, 0): "P", (0, 1): "D", (1, 1): "P"}
                    p1_late = set()
                    # GpSimd units stored first so batch 1's o-tile reuse
                    # (WAR on these stores) clears early
                    unit_order = [(1, 0), (1, 1), (0, 0), (0, 1)]
                vy = work.tile([MIP, 2 * H], F32, name=f"v_{b}", tag="v")
                mlp_ps = psum_mlp.tile([MIP, 2 * H], F32, name=f"mlp_{b}", tag=f"mlp{b % 2}")
                ah = {}
                aw_bf = {}
                tbf_t = {}
                for k in range(NCHUNK):
                    ah[k] = work.tile([128, H], F32, name=f"ah_{b}_{k}", tag=f"ah_{k}")
                    aw_bf[k] = work.tile([128, W], BF16, name=f"awbf_{b}_{k}", tag=f"awbf_{k}")

                def emit_p1(k, j):
                    t = tbf.tile([128, HCOL], BF16, name=f"t_{b}_{k}_{j}", tag=f"t{k}{j}")
                    tbf_t[k, j] = t
                    xs_ = xt[b, k, j][:].rearrange("p (h w) -> p h w", h=32)
                    ahb = ah[k][:, 32 * j:32 * j + 32].unsqueeze(2).broadcast_to((128, 32, W))
                    tv = t[:].rearrange("p (h w) -> p h w", h=32)
                    eng = nc.vector if p1_eng[k, j] == "D" else nc.gpsimd
                    eng.tensor_tensor(tv, xs_, ahb, Alu.mult)

                def emit_mlp(ps_slice, v_slice, u, m6):
                    # u = relu(y + bias + 3); v = (u-3)*min(u,6)/6 == hswish(BN(y))
                    nc.scalar.activation(u[:], ps_slice, Act.Relu, bias=bias3_p, scale=1.0)
                    nc.vector.tensor_scalar(m6[:], u[:], 6.0, 1.0 / 6.0, Alu.min, Alu.mult)
                    nc.vector.scalar_tensor_tensor(v_slice, u[:], -3.0, m6[:],
                                                   Alu.add, Alu.mult)

                for j in range(2):
                    # conv for half j: psum tile [64, 1024]; quarter q of this
                    # half at partition base 32q; 512-col slices accumulate
                    # x0, r0, x1, r1 (x+res for free).
                    y_ps = psum_y.tile([64, QCOL], F32, name=f"y_{b}_{j}", tag=f"y{j}")
                    srcs = [(0, xt[b, 0, j]), (0, rt[b, 0, j]),
                            (1, xt[b, 1, j]), (1, rt[b, 1, j])]
                    for s in range(2):
                        accum = [(q, k, src) for (k, src) in srcs for q in range(2)]
                        for i, (q, k, src) in enumerate(accum):
                            col = q * QCOL + s * 512
                            nc.tensor.matmul(
                                y_ps[0:64, s * 512:(s + 1) * 512],
                                w1m[k][q][:],
                                src[:, col:col + 512].bitcast(F32R),
                                start=(i == 0), stop=(i == len(accum) - 1),
                            )
                    # directional pools (DVE): free dim is just 1024.  The
                    # a_w path is the batch's critical path, so the column
                    # pool goes first.
                    ywp_s = work.tile([64, W], F32, name=f"ywp_{b}_{j}", tag=f"ywp{j}")
                    nc.vector.reduce_sum(
                        ywp_s[:], y_ps[:].rearrange("p (h w) -> p w h", h=QH), axis=AX.X)
                    # yw gather-sum accumulates across halves
                    nc.tensor.matmul(mlp_ps[:, H:2 * H], s32[0:64, :], ywp_s[:],
                                     start=(j == 0), stop=(j == 1))
                    yh_s = work.tile([64, QH], F32, name=f"yh_{b}_{j}", tag=f"yh{j}")
                    nc.vector.reduce_sum(
                        yh_s[:], y_ps[:].rearrange("p (h w) -> p h w", h=QH), axis=AX.X)
                    if j == 1:
                        # a_w chain as soon as the last column pool lands
                        ws_ = slice(H, 2 * H)
                        uw = work.tile([MIP, W], F32, name=f"uw_{b}", tag="uw")
                        m6w = work.tile([MIP, W], F32, name=f"m6w_{b}", tag="m6w")
                        emit_mlp(mlp_ps[:, ws_], vy[:, ws_], uw, m6w)
                        for k in range(NCHUNK):
                            cs = slice(k * 128, (k + 1) * 128)
                            awp = psum_att.tile([128, W], F32, name=f"awp_{b}_{k}", tag=f"att{k}")
                            nc.tensor.matmul(awp[:], w3t[:, cs], vy[:, ws_],
                                             start=True, stop=True)
                            nc.scalar.activation(aw_bf[k][:], awp[:], Act.Sigmoid,
                                                 bias=b3t[:, k:k + 1], scale=1.0)
                            nc.scalar.mul(aw_bf[k][:], aw_bf[k][:], 2.0)
                    # gather yh quarters -> mlp psum cols [32j, 32j+32)
                    # gather quarter q's rows via an identity column-slice
                    # stationary at base 0 (partition-offset operands are
                    # rejected by the hardware ISA)
                    for q in range(2):
                        nc.tensor.matmul(
                            mlp_ps[:, 32 * j + QH * q: 32 * j + QH * (q + 1)],
                            ident[0:64, 32 * q:32 * q + MIP],
                            yh_s[:, :],
                            start=True, stop=True)
                    # mlp + a_h for this half
                    hs = slice(32 * j, 32 * j + 32)
                    u = work.tile([MIP, 32], F32, name=f"u_{b}_{j}", tag=f"u{j}")
                    m6 = work.tile([MIP, 32], F32, name=f"m6_{b}_{j}", tag=f"m6{j}")
                    emit_mlp(mlp_ps[:, hs], vy[:, hs], u, m6)
                    for k in range(NCHUNK):
                        cs = slice(k * 128, (k + 1) * 128)
                        ahp = psum_att.tile([128, 32], F32, name=f"ahp_{b}_{j}_{k}",
                                            tag=f"att{(2 * j + k) % 2}")
                        nc.tensor.matmul(ahp[:], w2t[:, cs], vy[:, hs], start=True, stop=True)
                        nc.scalar.activation(ah[k][:, hs], ahp[:], Act.Sigmoid,
                                             bias=b2t[:, k:k + 1], scale=1.0)
                    # p1: t = (2x) * ah (f32 -> bf16)
                    for k in range(NCHUNK):
                        if (k, j) not in p1_late:
                            emit_p1(k, j)

                # finals: p2 on DVE (bf16 2x) and p3 + store, at half-tile
                # granularity so the output stream starts right after a_w
                # lands.  The critical batch uses a hand-interleaved order so
                # DVE and GpSimd finish together and every store meets its
                # DMA slot.
                ot = {}
                for j in range(2):
                    for k in range(NCHUNK):
                        # the last batch's outputs reuse batch 0's x buffers
                        # (dead after batch 0's conv+p1) to avoid WAR stalls
                        # against batch 0's outgoing stores
                        tag = f"x0{k}{j}" if b == LAST else f"o{k}{j}"
                        ot[k, j] = big.tile([128, HCOL], F32, name=f"o_{b}_{k}_{j}",
                                            tag=tag)

                def p2_half(k, j, hhalf):
                    t = tbf_t[k, j]
                    sl = slice(hhalf * 1024, (hhalf + 1) * 1024)
                    tv = t[:, sl].rearrange("p (h w) -> p h w", h=16)
                    awb = aw_bf[k][:].unsqueeze(1).broadcast_to((128, 16, W))
                    nc.vector.tensor_tensor(tv, tv, awb, Alu.mult)

                def pre2r(k, j):
                    # o = 2r for GpSimd p3 units (Act has slack; GpSimd lacks
                    # a fused scalar_tensor_tensor on hardware)
                    nc.scalar.mul(ot[k, j][:], rt[b, k, j][:], 2.0)

                def p3_store(k, j, hhalf):
                    t = tbf_t[k, j]
                    r = rt[b, k, j]
                    o = ot[k, j]
                    sl = slice(hhalf * 1024, (hhalf + 1) * 1024)
                    if p3_eng[k, j] == "D":
                        nc.vector.scalar_tensor_tensor(
                            o[:, sl], r[:, sl], 2.0, t[:, sl], Alu.mult, Alu.add)
                    else:
                        nc.gpsimd.tensor_tensor(o[:, sl], o[:, sl], t[:, sl], Alu.add)
                    od = out_d[b, k * 128:(k + 1) * 128].rearrange("c h w -> c (h w)")
                    nc.sync.dma_start(
                        od[:, j * HCOL + sl.start: j * HCOL + sl.stop], o[:, sl])

                for (kk, jj), e in p3_eng.items():
                    if e != "D":
                        pre2r(kk, jj)
                if b == LAST:
                    p2_half(0, 0, 0); p3_store(0, 0, 0)
                    p2_half(0, 0, 1); p3_store(0, 0, 1)
                    p2_half(1, 0, 0); p3_store(1, 0, 0)
                    p2_half(1, 0, 1); p3_store(1, 0, 1)
                    p2_half(1, 1, 0); p2_half(1, 1, 1)
                    p3_store(1, 1, 0)
                    emit_p1(0, 1)
                    p2_half(0, 1, 0); p3_store(0, 1, 0)
                    p3_store(1, 1, 1)
                    p2_half(0, 1, 1); p3_store(0, 1, 1)
                else:
                    for k, j in unit_order:
                        for hhalf in range(2):
                            p2_half(k, j, hhalf)
                            p3_store(k, j, hhalf)

    nc.compile()
    return nc


_NC_CACHE = None


def _get_module():
    global _NC_CACHE
    if _NC_CACHE is None:
        _NC_CACHE = build_module()
    return _NC_CACHE


def make_in_maps(inputs):
    reps = {k: np.ascontiguousarray(v) for k, v in inputs.items()
            if k not in ("x", "residual")}
    in_maps = []
    for core in range(N_CORES):
        bs = slice(core * NLOC, (core + 1) * NLOC)
        m = {"x": np.ascontiguousarray(inputs["x"][bs]),
             "residual": np.ascontiguousarray(inputs["residual"][bs])}
        m.update(reps)
        in_maps.append(m)
    return in_maps


def run_spmd(nc, in_maps):
    res = run_bass_kernel_spmd(nc, in_maps, core_ids=list(range(N_CORES)))
    return np.concatenate([res.results[c]["out"] for c in range(N_CORES)], axis=0)


def kernel(**inputs):
    inputs = {k: np.asarray(v) for k, v in inputs.items()}
    nc = _get_module()
    return run_spmd(nc, make_in_maps(inputs))
